# revision 50
# baseline (speedup 1.0000x reference)
"""Median graph convolution on 8 Trainium2 NeuronCores.

out[n, c] = median over valid neighbors j of (x @ kernel)[neighbors[n, j], c]
(lower median, rank (deg-1)//2 of the first deg neighbor slots).

Strategy (data-parallel over nodes, 6272 nodes/core):
  - host sorts nodes by degree (descending), striped across the 8 cores so
    every core sees the same degree profile and one compiled program fits all
  - each core matmuls its node shard on the PE -> h shard (fp16),
    AllGather into a per-core HBM table with trailing +inf sentinel rows
  - the table is indexed as 512-byte PAIR rows (two h rows per descriptor),
    so the 50176-row table needs only 25089 int16-indexable pair rows;
    each real neighbor costs exactly ONE gather descriptor
  - only the first maxdeg(tile) slots are gathered per 128-node tile
    (pads ride as +inf sentinel descriptors / strided memset)
  - copy_predicated (int16 parity mask, stride-0 broadcast over channels)
    selects the wanted half of each gathered pair IN PLACE onto the a-half;
    sort stage 0 then reads the a-halves with a 512B slot stride, so no
    separate value-copy pass is needed
  - a degree-adaptive bitonic network sorts the two H-halves of the slot
    array and a rank-r two-way merge formula extracts the lower median
"""

import sys

sys.path.insert(0, "/opt/trn_rl_repo")

import numpy as np

N, K, IN_C, OUT_C = 50000, 32, 256, 128
NCORES = 8
NTILES = 49                      # 128-node tiles per core
SHARD = NTILES * 128             # 6272
NPAD = SHARD * NCORES            # 50176
TROWS = NPAD + 8                 # +inf / -inf sentinel rows at the end
SENT_PAIR = NPAD // 2            # pair index of the +inf sentinel rows
SENT_NEG = NPAD // 2 + 2         # pair index of the -inf sentinel rows
NPAIRS = SENT_NEG + 1            # pair rows addressable by the gather
GCHUNK = 8                       # slots per dma_gather call (8*128 = 1024 idx)
MAXSLOTS = 32
AGBOUNDS = (0, NTILES)           # single AllGather chunk: the warmup hides
                                 # the barrier, and each CC op costs ~10us
                                 # fixed, so chunking only adds overhead

_CACHE = {}


def _next_pow2(x):
    p = 1
    while p < x:
        p *= 2
    return p


def _make_schedule(deg_sorted):
    """Per-tile (maxd, H, r) from the global descending degree profile.

    Every node in a tile is rank-pinned to the tile's max rank r by padding
    lower-rank nodes with -inf sentinels (one per missing rank)."""
    sched = []
    for t in range(NTILES):
        degs = deg_sorted[t * 128 * NCORES:(t + 1) * 128 * NCORES]
        maxd = int(degs[0])
        H = max(1, _next_pow2(maxd) // 2)
        sched.append((maxd, H, int((maxd - 1) // 2)))
    return tuple(sched)


def _emit_program(sched):
    import concourse.tile as tile
    import concourse.mybir as mybir
    from concourse import bacc
    from concourse.bass import AP
    from concourse.library_config import mlp

    fp16 = mybir.dt.float16
    fp32 = mybir.dt.float32
    i16 = mybir.dt.int16
    Alu = mybir.AluOpType

    tot_idx_cols = sum(s[0] * 8 for s in sched)
    tot_par_cols = sum(s[0] for s in sched)

    nc = bacc.Bacc("TRN2", target_bir_lowering=False, num_swdge_queues=4,
                   dynamic_dma_scratch_size=32768)

    xT = nc.dram_tensor("xT", [IN_C, SHARD], fp16, kind="ExternalInput")
    w = nc.dram_tensor("w", [IN_C, OUT_C], fp16, kind="ExternalInput")
    idx_d = nc.dram_tensor("idx", [128, tot_idx_cols], i16, kind="ExternalInput")
    par_d = nc.dram_tensor("par", [128, tot_par_cols], i16, kind="ExternalInput")
    infs = nc.dram_tensor("infs", [8, OUT_C], fp16, kind="ExternalInput")  # +/-inf
    out = nc.dram_tensor("out", [SHARD, OUT_C], fp16, kind="ExternalOutput")
    table = nc.dram_tensor("table", [TROWS, OUT_C], fp16, addr_space="Shared")
    hshard = nc.dram_tensor("hshard", [SHARD, OUT_C], fp16)

    # gather source: the table viewed as 512B pair rows [NPAIRS, 256]
    pair_ap = AP(table[:].tensor, 0, [[2 * OUT_C, NPAIRS], [1, 2 * OUT_C]])

    S = OUT_C       # slot stride (elements) in a compact value tile
    SB = 2 * OUT_C  # slot stride of the a-halves inside the raw pair buffer

    def slot_ap(t, slot0, dims, stride=None):
        """AP over value tile t: partition dim + (slot_step, count) dims + c.

        stride overrides the slot stride in elements (SB for the raw pair
        buffer whose a-halves act as the stage-0 value array)."""
        ss = S if stride is None else stride
        base = t[:]
        free = [[st * ss, ct] for (st, ct) in dims if ct != 1]
        return AP(base.tensor, base.offset + slot0 * ss, [base.ap[0]] + free + [[1, OUT_C]])

    def stages_for(top):
        """(k, j, allasc) stage list; allasc on the final k group."""
        ks = []
        k = 2
        while k <= top:
            j = k // 2
            while j >= 1:
                ks.append((k, j, k == top))
                j //= 2
            k *= 2
        return ks

    with tile.TileContext(nc) as tc:
        # NOTE: no warmup collective — traces show the CC rendezvous barrier
        # self-initiates at ~21us regardless of when the first CC op is
        # kicked, so a warmup op only serializes ~10us in front of the real
        # AllGather on the CC stream.
        nc.gpsimd.load_library(mlp)
        with (
            tc.tile_pool(name="const", bufs=1) as cpool,
            tc.tile_pool(name="psum", bufs=2, space="PSUM") as psum_pool,
            tc.tile_pool(name="gbuf", bufs=6) as gpool,
            tc.tile_pool(name="work", bufs=2) as wpool,
            tc.tile_pool(name="mout", bufs=2) as mpool,
        ):
            # ---- phase 1+2: h rows = x @ w (x chunk stationary -> [node, c]),
            # AllGather pipelined in chunks behind the matmul ----
            inft = cpool.tile([8, OUT_C], fp16)
            nc.sync.dma_start(inft[:], infs[:])
            nc.sync.dma_start(table[NPAD:NPAD + 8, :], inft[:])
            with tc.tile_pool(name="stage", bufs=1) as spool:
                lw0 = spool.tile([128, OUT_C], fp16)
                lw1 = spool.tile([128, OUT_C], fp16)
                nc.sync.dma_start(lw0[:], w[0:128, :])
                nc.sync.dma_start(lw1[:], w[128:256, :])
                xt0 = spool.tile([128, SHARD], fp16)
                xt1 = spool.tile([128, SHARD], fp16)
                nc.sync.dma_start(xt0[:], xT[0:128, :])
                nc.sync.dma_start(xt1[:], xT[128:256, :])
                hrows = spool.tile([128, NTILES, OUT_C], fp16)
                for j in range(NTILES):
                    ns = slice(j * 128, (j + 1) * 128)
                    ps = psum_pool.tile([128, OUT_C], fp32)
                    nc.tensor.matmul(ps[:], lhsT=xt0[:, ns], rhs=lw0[:], start=True, stop=False)
                    nc.tensor.matmul(ps[:], lhsT=xt1[:, ns], rhs=lw1[:], start=False, stop=True)
                    nc.scalar.copy(hrows[:, j, :], ps[:])
                    if j + 1 in AGBOUNDS[1:]:
                        ci = AGBOUNDS.index(j + 1)
                        c0 = AGBOUNDS[ci - 1]
                        nc.sync.dma_start(
                            hshard[c0 * 128:(j + 1) * 128, :].rearrange(
                                "(j n) c -> n j c", n=128),
                            hrows[:, c0:j + 1, :],
                        )
                        nc.gpsimd.collective_compute(
                            "AllGather",
                            mybir.AluOpType.bypass,
                            replica_groups=[list(range(NCORES))],
                            ins=[hshard[c0 * 128:(j + 1) * 128, :]],
                            outs=[table[c0 * 128 * NCORES:(j + 1) * 128 * NCORES, :]],
                        )

            # ---- load index/mask streams; +inf constant for pad slots ----
            idx_sb = cpool.tile([128, tot_idx_cols], i16)
            par_sb = cpool.tile([128, tot_par_cols], i16)
            nc.sync.dma_start(idx_sb[:], idx_d[:])
            nc.sync.dma_start(par_sb[:], par_d[:])

            # ---- phase 3: gather + select + sort + median per tile ----
            # interleave heavy (high-maxd) and light tiles so descgen on
            # GpSimd and the sort on Vector stay rate-matched instead of
            # alternating between gpsimd-bound and vector-bound phases
            icols, pcols = [], []
            ic = pc = 0
            for s in sched:
                icols.append(ic)
                pcols.append(pc)
                ic += s[0] * 8
                pc += s[0]
            # lead with a mid-size (2-gather-chunk) tile so the vector engine
            # starts ~8us after the table lands, then zip heavies (0..23)
            # with mids (25..48): mid tiles carry ~3us of vector work per
            # gather chunk (vs ~1us for the trivial tail tiles), which covers
            # the heavy tiles' descriptor generation; the cheapest tiles fall
            # at the end, shortening the drain tail.
            mid = NTILES // 2
            rest = [t for t in range(NTILES) if t != mid]
            heavies, lights = rest[:len(rest) // 2], rest[len(rest) // 2:]
            order = [mid]
            for a, b in zip(heavies, lights):
                order += [a, b]
            order += heavies[len(lights):] + lights[len(heavies):]

            # pre-generate gather descriptors for the first two processed
            # tiles during the AllGather wait (desc-gen has no table
            # dependency; the deferred table read gates trigger_dma instead),
            # so their transfers fire the moment the table is complete
            qn = 0        # dma queue rotation
            psems = [nc.alloc_semaphore(f"pregather{q}") for q in range(4)]
            prebufs = {}
            prepwaits = {}
            pq_count = [0] * 4
            ntrig = [0] * 4
            for t in order[:0]:   # prep-ahead: no measurable gain; disabled
                maxd_t = sched[t][0]
                b = gpool.tile([128, MAXSLOTS, 2 * OUT_C], fp16, tag="pair")
                prebufs[t] = b
                for s0 in range(0, maxd_t, GCHUNK):
                    s1 = min(s0 + GCHUNK, maxd_t)
                    G = (s1 - s0) * 128
                    nc.gpsimd.dma_gather(
                        b[:, s0:s1, :],
                        pair_ap,
                        idx_sb[:, icols[t] + s0 * 8: icols[t] + s1 * 8],
                        G, G, 2 * OUT_C,
                        queue_num=qn, single_packet=False,
                        prepare_only=True, sem=psems[qn])
                    pq_count[qn] += 16
                    prepwaits.setdefault(t, []).append((psems[qn], pq_count[qn]))
                    ntrig[qn] += 1
                    qn = (qn + 1) % 4
            for q in range(4):
                if ntrig[q]:
                    nc.gpsimd.trigger_dma(count=None, queue_num=q)

            for t in order:
                maxd, H, r, elo, ehi = sched[t]
                icol = icols[t]
                pcol = pcols[t]
                P2 = 2 * H
                HRr = maxd - H                      # real R-half values
                HR = 0 if HRr < 1 else _next_pow2(HRr)
                fused = H >= 2 and HR == H
                span = P2 if fused else H + HR      # slots read by the nets

                buf = prebufs.pop(t, None)
                if buf is None:
                    buf = gpool.tile([128, MAXSLOTS, 2 * OUT_C], fp16, tag="pair")
                    for s0 in range(0, maxd, GCHUNK):
                        s1 = min(s0 + GCHUNK, maxd)
                        G = (s1 - s0) * 128
                        nc.gpsimd.dma_gather(
                            buf[:, s0:s1, :],
                            pair_ap,
                            idx_sb[:, icol + s0 * 8: icol + s1 * 8],
                            G, G, 2 * OUT_C,
                            queue_num=qn, single_packet=False)
                        qn = (qn + 1) % 4

                # select the wanted half of each pair IN PLACE onto the
                # a-half. Slots are parity-sorted per node, so only the
                # mixed band [elo, ehi) needs the (1x-rate) predicated copy;
                # the all-odd tail [ehi, maxd) is a plain b->a copy on the
                # otherwise-idle Scalar engine; the all-even head is free.
                bb = buf[:]
                # prepped tiles: the DMA completion sem is user-synced — gate
                # the vector stream on every chunk's completion explicitly
                for sem_, val_ in prepwaits.get(t, ()):
                    nc.vector.wait_ge(sem_, val_)
                if ehi > elo:
                    mix = ehi - elo
                    a_ap = AP(bb.tensor, bb.offset + elo * SB,
                              [bb.ap[0], [SB, mix], [1, OUT_C]])
                    b_ap = AP(bb.tensor, bb.offset + elo * SB + OUT_C,
                              [bb.ap[0], [SB, mix], [1, OUT_C]])
                    pp = par_sb[:]
                    m_ap = AP(pp.tensor, pp.offset + pcol + elo,
                              [pp.ap[0], [1, mix], [0, OUT_C]])
                    nc.vector.copy_predicated(a_ap, m_ap, b_ap)
                if maxd > ehi:
                    tl = maxd - ehi
                    nc.scalar.copy(
                        AP(bb.tensor, bb.offset + ehi * SB,
                           [bb.ap[0], [SB, tl], [1, OUT_C]]),
                        AP(bb.tensor, bb.offset + ehi * SB + OUT_C,
                           [bb.ap[0], [SB, tl], [1, OUT_C]]),
                    )
                if maxd < span:
                    nc.gpsimd.memset(
                        slot_ap(buf, maxd, [(1, span - maxd)], stride=SB),
                        float("inf"))

                v0 = wpool.tile([128, MAXSLOTS, OUT_C], fp16, tag="v0")
                v1 = wpool.tile([128, MAXSLOTS, OUT_C], fp16, tag="v1")

                def emit_net(stages, base, W, first=None):
                    """Bitonic network over slots [base, base+W); stage 0
                    reads the buf a-halves (512B stride), later stages
                    ping-pong v0/v1 starting at `first`. Returns
                    (tile, slot_stride) of the final values of that region."""
                    cur, cstr = buf, SB
                    nxt = v0 if first is None else first
                    for (k, j, allasc) in stages:
                        if allasc:
                            lo = [(2 * j, W // (2 * j)), (1, j)]
                            for op, off in ((Alu.min, 0), (Alu.max, j)):
                                nc.vector.tensor_tensor(
                                    out=slot_ap(nxt, base + off, lo),
                                    in0=slot_ap(cur, base, lo, stride=cstr),
                                    in1=slot_ap(cur, base + j, lo, stride=cstr),
                                    op=op,
                                )
                        elif (
                            # DVE ISA: at most 3 free dims after the opt pass
                            # merges a contiguous (1, j)+channel tail (merge
                            # happens only when the slot stride is S).
                            max(
                                (W > 2 * k) + 1 + (k > 2 * j) + 1
                                + (1 if (j > 1 and cstr != S) else 0),  # in
                                (W > 2 * k) + 1 + (k > 2 * j) + 1,      # out
                            ) <= 3
                        ):
                            # each 2k period holds an ascending block at +0
                            # and a descending block at +k; one min and one
                            # max TT cover both (outputs swap to +j / +0 in
                            # the descending block via the (k±j, 2) dim).
                            # Only when a count-1 dim drops out: the DVE AP
                            # supports at most 4 free dims incl. channels.
                            din = [(2 * k, W // (2 * k)), (k, 2),
                                   (2 * j, k // (2 * j)), (1, j)]
                            nc.vector.tensor_tensor(
                                out=slot_ap(nxt, base,
                                            [(2 * k, W // (2 * k)), (k + j, 2),
                                             (2 * j, k // (2 * j)), (1, j)]),
                                in0=slot_ap(cur, base, din, stride=cstr),
                                in1=slot_ap(cur, base + j, din, stride=cstr),
                                op=Alu.min,
                            )
                            nc.vector.tensor_tensor(
                                out=slot_ap(nxt, base + j,
                                            [(2 * k, W // (2 * k)), (k - j, 2),
                                             (2 * j, k // (2 * j)), (1, j)]),
                                in0=slot_ap(cur, base, din, stride=cstr),
                                in1=slot_ap(cur, base + j, din, stride=cstr),
                                op=Alu.max,
                            )
                        else:
                            dims = [(2 * k, W // (2 * k)), (2 * j, k // (2 * j)), (1, j)]
                            for desc in (0, 1):
                                b0 = base + (k if desc else 0)
                                lo_out, hi_out = (j, 0) if desc else (0, j)
                                nc.vector.tensor_tensor(
                                    out=slot_ap(nxt, b0 + lo_out, dims),
                                    in0=slot_ap(cur, b0, dims, stride=cstr),
                                    in1=slot_ap(cur, b0 + j, dims, stride=cstr),
                                    op=Alu.min,
                                )
                                nc.vector.tensor_tensor(
                                    out=slot_ap(nxt, b0 + hi_out, dims),
                                    in0=slot_ap(cur, b0, dims, stride=cstr),
                                    in1=slot_ap(cur, b0 + j, dims, stride=cstr),
                                    op=Alu.max,
                                )
                        cur, cstr = nxt, S
                        nxt = v1 if nxt is v0 else v0
                    return cur, cstr

                if fused:
                    # R needs the full depth: fused both-halves network
                    fin, fstr = emit_net(stages_for(H), 0, P2)
                else:
                    # start the R net on whichever scratch tile makes its
                    # final stage land in the same tile as the L net's
                    nL = len(stages_for(H)) if H >= 2 else 0
                    nR = len(stages_for(HR)) if HRr >= 2 else 0
                    L_end = v0 if nL % 2 == 1 else v1
                    R_first = L_end if nR % 2 == 1 else (v1 if L_end is v0 else v0)
                    finR, frstr = (emit_net(stages_for(HR), H, HR, first=R_first)
                                   if HRr >= 2 else (buf, SB))
                    fin, fstr = (emit_net(stages_for(H), 0, H)
                                 if H >= 2 else (buf, SB))
                    if (finR is not fin or frstr != fstr) and HR > 0:
                        nc.vector.tensor_copy(
                            slot_ap(fin, H, [(1, HR)], stride=fstr),
                            slot_ap(finR, H, [(1, HR)], stride=frstr),
                        )

                # L sorted in fin[0:H], R sorted in fin[H:H+HR] (+inf beyond);
                # every node is rank-pinned to the tile rank r
                o16 = mpool.tile([128, OUT_C], fp16, tag="o16")
                m = mpool.tile([128, K // 2 + 1, OUT_C], fp16, tag="m0")
                sv = fin[:]
                t0 = max(0, r - HR)
                nc1 = r - t0                        # max(L[t], R[r-1-t]) cands
                if nc1 > 0:
                    nc.vector.tensor_tensor(
                        out=slot_ap(m, 0, [(1, nc1)]),
                        in0=slot_ap(fin, t0, [(1, nc1)], stride=fstr),
                        in1=AP(sv.tensor, sv.offset + (H + r - 1 - t0) * fstr,
                               [sv.ap[0], [-fstr, nc1], [1, OUT_C]]),
                        op=Alu.max,
                    )
                # min-reduce cands[0..nc1), then fold in the boundary
                # candidates L[r] (always) and R[r] (when r < HR) directly
                n = nc1
                while n > 1:
                    a = n - n // 2
                    nc.vector.tensor_tensor(
                        out=slot_ap(m, 0, [(1, n // 2)]),
                        in0=slot_ap(m, 0, [(1, n // 2)]),
                        in1=slot_ap(m, a, [(1, n // 2)]),
                        op=Alu.min,
                    )
                    n = a
                if nc1 > 0:
                    nc.vector.tensor_tensor(
                        out=o16[:],
                        in0=slot_ap(m, 0, [(1, 1)]),
                        in1=slot_ap(fin, r, [(1, 1)], stride=fstr),
                        op=Alu.min,
                    )
                elif r < HR:
                    nc.vector.tensor_tensor(
                        out=o16[:],
                        in0=slot_ap(fin, r, [(1, 1)], stride=fstr),
                        in1=slot_ap(fin, H + r, [(1, 1)], stride=fstr),
                        op=Alu.min,
                    )
                else:
                    nc.vector.tensor_copy(
                        o16[:], slot_ap(fin, r, [(1, 1)], stride=fstr))
                if nc1 > 0 and r < HR:
                    nc.vector.tensor_tensor(
                        out=o16[:],
                        in0=o16[:],
                        in1=slot_ap(fin, H + r, [(1, 1)], stride=fstr),
                        op=Alu.min,
                    )

                nc.sync.dma_start(out[t * 128:(t + 1) * 128, :], o16[:])

    nc.compile()
    return nc


def _prepare(x, kernel, neighbors, degrees):
    """Host-side marshaling: permutation, schedule, idx/mask streams."""
    deg = np.clip(np.asarray(degrees).astype(np.int64), 1, K)
    deg_pad = np.ones(NPAD, np.int64)
    deg_pad[:N] = deg
    # dummies (N..NPAD) have deg 1 but gather only sentinels
    order = np.argsort(-deg_pad, kind="stable")        # global rank -> node id

    karr = np.arange(K, dtype=np.int64)[None, :]
    nbr = np.asarray(neighbors).astype(np.int64)

    # Table row of a node on core c, local slot i follows the chunk-major
    # layout row(c, i) = chunk_base*8 + c*chunk_rows + (i - chunk_base).
    # All chunk bases and core strides are EVEN, so row parity == slot
    # parity. Under the plain striping (rank j -> core j%8, slot j//8) a
    # node's row parity is therefore (j//8) & 1, which lets us compute each
    # node's even-row-neighbor count e BEFORE choosing the final placement.
    ranks = np.empty(NPAD, np.int64)
    ranks[order] = np.arange(NPAD)
    par0 = (ranks // NCORES) & 1                       # striped row parity
    e_pad = np.ones(NPAD, np.int64)                    # dummies: e = 1
    e_pad[:N] = ((karr < deg[:, None]) & (par0[nbr] == 0)).sum(1)

    # Parity-preserving (d, e) placement: nodes that sat on even slots stay
    # on even slots and likewise for odd — every node keeps its row PARITY
    # (so e stays valid). Each parity lane is sorted GLOBALLY by degree desc
    # then e in snake order (alternating direction per degree class keeps e
    # continuous across class boundaries) and striped across the cores, so
    # all cores' tile-t windows cover the same (d, e) quantile range and the
    # cross-core union predication band stays narrow.
    all_ids = np.arange(NPAD)
    lanes = []
    for p in (0, 1):
        lane = all_ids[par0 == p]
        ekey = np.where(deg_pad[lane] % 2 == 0, -e_pad[lane], e_pad[lane])
        lanes.append(lane[np.lexsort((ekey, -deg_pad[lane]))])
    placements = []
    for c in range(NCORES):
        newn = np.empty(SHARD, np.int64)
        newn[0::2] = lanes[0][c::NCORES]
        newn[1::2] = lanes[1][c::NCORES]
        placements.append(newn)

    bounds = np.array(AGBOUNDS, np.int64) * 128        # local row boundaries
    loc_arr = np.arange(SHARD)
    chunk_l = np.searchsorted(bounds, loc_arr, side="right") - 1
    csz_l = bounds[chunk_l + 1] - bounds[chunk_l]
    base_l = bounds[chunk_l]
    rho = np.empty(NPAD, np.int64)
    for c in range(NCORES):
        rho[placements[c]] = base_l * NCORES + c * csz_l + (loc_arr - base_l)

    # schedule from the actual per-tile degree maxima (cross-core max; the
    # parity split can shift per-tile windows by +-1 vs the striped order)
    dmat = np.stack([deg_pad[p] for p in placements])  # [8, SHARD]
    tmax = dmat.reshape(NCORES, NTILES, 128).max(axis=(0, 2))
    sched = tuple(
        (int(m), max(1, _next_pow2(int(m)) // 2), int((m - 1) // 2))
        for m in tmax
    )

    nbr_rows = rho[nbr]                                # [N, K]
    pair_full = np.zeros((NPAD, K), np.int64)
    par_full = np.zeros((NPAD, K), np.int64)
    pair_full[:N] = nbr_rows >> 1
    par_full[:N] = nbr_rows & 1

    xf = np.zeros((NPAD, IN_C), np.float16)
    xf[:N] = np.asarray(x, np.float32).astype(np.float16)
    wf = np.asarray(kernel, np.float32).astype(np.float16)
    infs = np.concatenate([np.full((4, OUT_C), np.inf, np.float16),
                           np.full((4, OUT_C), -np.inf, np.float16)])

    in_maps = []
    node_of = np.empty((NCORES, SHARD), np.int64)
    elo_all = [K + 1] * NTILES
    ehi_all = [0] * NTILES
    for c in range(NCORES):
        nodes_c = placements[c]                        # local slot i -> node id
        node_of[c] = nodes_c
        d_c = deg_pad[nodes_c]                         # desc per parity lane
        pair_c = pair_full[nodes_c]                    # [SHARD, K]
        par_c = par_full[nodes_c]
        valid_c = karr < d_c[:, None]                  # [SHARD, K]

        idx_parts = []
        par_parts = []
        for t, (maxd, H, r) in enumerate(sched):
            sl = slice(t * 128, (t + 1) * 128)
            pt = pair_c[sl, :maxd]                     # [128, maxd]
            vt = valid_c[sl, :maxd]
            d_t = d_c[sl]                              # [128]
            # rank-pin: nodes below the tile rank get -inf pads right after
            # their real values (one per missing rank); +inf beyond
            p_t = r - (d_t - 1) // 2                   # [128] pads needed
            assert np.all(d_t + p_t <= maxd)
            neg = karr[:, :maxd] < (d_t + p_t)[:, None]
            stream_full = np.where(
                vt, pt, np.where(neg, SENT_NEG, SENT_PAIR)
            )                                          # [128, maxd]
            # reorder each node's slots: even-parity neighbors first, then
            # odd, pads last (median is order-invariant). The parity mask
            # becomes a per-node prefix pattern, so only the mixed band
            # [e_lo, e_hi) needs copy_predicated. Low-e nodes additionally
            # pull some of their parity-agnostic sentinel pads into the
            # head, raising e_lo to min(e + pads) and narrowing the band.
            parv = np.where(vt, par_c[sl, :maxd], 0)
            key = np.where(vt, parv, 2)
            e_cnt = (key == 0).sum(axis=1)             # evens per node
            avail = maxd - d_t                         # pads per node
            cap = int((e_cnt + avail).min())
            head_need = np.clip(cap - e_cnt, 0, avail)
            pad_ord = np.cumsum(key == 2, axis=1) - (key == 2)
            key = np.where(
                key == 2,
                np.where(pad_ord < head_need[:, None], 1, 3),
                np.where(key == 0, 0, 2),
            )
            ordx = np.argsort(key, axis=1, kind="stable")
            stream = np.take_along_axis(stream_full, ordx, axis=1).T
            parv = np.take_along_axis(parv, ordx, axis=1)
            e_eff = e_cnt + head_need                  # effective head length
            elo_all[t] = min(elo_all[t], int(e_eff.min()))
            ehi_all[t] = max(ehi_all[t], int(e_eff.max()))
            wrapped = np.tile(
                stream.reshape(maxd * 8, 16).T, (8, 1)
            )                                          # [128, maxd*8]
            idx_parts.append(wrapped.astype(np.int16))
            par_parts.append(parv.astype(np.int16))    # [128, maxd]

        idx_all = np.ascontiguousarray(np.concatenate(idx_parts, axis=1))
        par_all = np.ascontiguousarray(np.concatenate(par_parts, axis=1))
        in_maps.append({
            "xT": np.ascontiguousarray(xf[nodes_c].T),
            "w": wf,
            "idx": idx_all,
            "par": par_all,
            "infs": infs,
        })

    sched_ext = tuple(
        (maxd, H, r, min(elo_all[t], maxd), max(ehi_all[t], min(elo_all[t], maxd)))
        for t, (maxd, H, r) in enumerate(sched)
    )
    return sched_ext, in_maps, node_of


def kernel(x, kernel, neighbors, degrees):
    from concourse.bass_utils import run_bass_kernel_spmd

    sched, in_maps, node_of = _prepare(x, kernel, neighbors, degrees)
    if sched not in _CACHE:
        _CACHE[sched] = _emit_program(sched)
    nc = _CACHE[sched]

    res = run_bass_kernel_spmd(nc, in_maps, list(range(NCORES)))
    full = np.empty((NPAD, OUT_C), np.float32)
    for c in range(NCORES):
        full[node_of[c]] = res.results[c]["out"].astype(np.float32)
    return np.ascontiguousarray(full[:N])
